# revision 18
# baseline (speedup 1.0000x reference)
"""Self-contained Trainium2 Bass kernel for nn_AttentionBlock
(B=2, N=2048, D=512, H=8, MLP 2x).

kernel(**inputs) takes the FULL unsharded inputs (as produced by
setup_inputs) and returns the FULL (2, 2048, 512) output.

Sharding: 2-way data-parallel over batch x 4-way parallel over
query-token slices (8 cores, no collectives).  Each core computes
K/V for its whole batch and attention + MLP for its 512-token slice.

v3: fp8(e4m3) DoubleRow matmuls (0.5 cycles/row, K=256/instr) for all
K>=256 GEMMs including softmax@V (ones-column denominator packed into
the fp8 V tile).  The Activation engine's softmax-exp stream (64 x
[128,1024], ~66us) is the roofline; the PE stream is interleaved into
the exp cadence: scores for kv-chunk jc+1 are emitted right behind
exp(jc), and V-proj / K-proj(m+1) / A@V(m-1) fill the per-period PE
slack so the exp pipe never starves.  LN1 stats are pipelined per
512-token tile; exp folds rstd_kv/64 as a per-partition scale AP;
K-bias rides the PSUM->SBUF copy (scalar_tensor_tensor); weights ship
as 8*W in fp8 with the 1/8 recovered in free scale slots.
"""

from contextlib import ExitStack

import numpy as np

import concourse.bass as bass
import concourse.mybir as mybir
import concourse.tile as tile

_WSPLIT_UID = [0]


def _finalize(nc, max_waits=1):
    """Split multi-sem-wait instructions onto single-wait NoOp carriers
    (the walrus build in this container accepts one wait per instruction)."""
    for f in nc.m.functions:
        for bb in f.blocks:
            insts = bb.instructions
            out = []
            changed = False
            for inst in insts:
                si = inst.sync_info
                waits = list(si.on_wait) if (si and si.on_wait) else []
                if len(waits) > max_waits:
                    changed = True
                    for w in waits[:-max_waits]:
                        _WSPLIT_UID[0] += 1
                        nop = mybir.InstNoOp(
                            name=f"I-wsplit-{_WSPLIT_UID[0]}",
                            ins=[], outs=[], engine=inst.engine,
                        )
                        nop.sync_info = mybir.SyncInfo(on_wait=[w],
                                                       on_update=[])
                        out.append(nop)
                    si.on_wait = waits[-max_waits:]
                out.append(inst)
            if changed:
                bb.instructions = out
    return nc


BF16 = mybir.dt.bfloat16
F32 = mybir.dt.float32
F32R = mybir.dt.float32r
FP8 = mybir.dt.float8e4
AF = mybir.ActivationFunctionType
OP = mybir.AluOpType
DR = mybir.MatmulPerfMode.DoubleRow

P = 128
B, N, D, H = 2, 2048, 512, 8
HD = D // H          # 64
TC = 512             # tokens per core
DM = 2 * D           # 1024 mlp hidden
KC = D // P          # 4 chunks of the 512 feature dim
NT = N // 512        # 4 column tiles of 512 over the 2048 kv tokens
JC = N // P          # 16 token chunks of 128 over kv tokens
MC1 = DM // P        # 8 chunks of mlp hidden
EPS = 1e-5
S = 8.0              # fp8 weight pre-scale


def dram_bcast_src(dram_ap, nparts):
    """AP re-reading a [1, n] DRAM row on `nparts` partitions (DMA src)."""
    return bass.AP(
        tensor=dram_ap.tensor,
        offset=dram_ap.offset,
        ap=[[0, nparts]] + [list(x) for x in dram_ap.ap[1:]],
    )


def build_nc(do_finalize=True):
    nc = bass.Bass()
    x8d = nc.dram_tensor("x8d", [D, N], FP8, kind="ExternalInput")
    yres = nc.dram_tensor("yres", [D, TC], F32, kind="ExternalInput")
    Wq8 = nc.dram_tensor("Wq8", [D, D], FP8, kind="ExternalInput")
    Wk8 = nc.dram_tensor("Wk8", [D, D], FP8, kind="ExternalInput")
    Wv8 = nc.dram_tensor("Wv8", [D, D], FP8, kind="ExternalInput")
    Wo8 = nc.dram_tensor("Wo8", [D, D], FP8, kind="ExternalInput")
    W1b = nc.dram_tensor("W1b", [D, DM], BF16, kind="ExternalInput")
    W28 = nc.dram_tensor("W28", [DM, D], FP8, kind="ExternalInput")
    W28r = nc.dram_tensor("W28r", [DM, D], FP8, kind="ExternalInput")
    # crow8 rows: 0 = 8*bq_, 1 = 8*(-sum Wq_), 2 = 8*(-sum Wv_)
    crow8 = nc.dram_tensor("crow8", [4, D], F32, kind="ExternalInput")
    wk8col = nc.dram_tensor("wk8col", [P, KC], F32, kind="ExternalInput")
    b2row8 = nc.dram_tensor("b2row8", [1, D], F32, kind="ExternalInput")
    ccol = nc.dram_tensor("ccol", [P, KC + MC1], F32, kind="ExternalInput")
    outT = nc.dram_tensor("outT", [D, TC], F32, kind="ExternalOutput")

    BQ, NSQ, NSV = 0, 1, 2  # crow8 rows

    with tile.TileContext(nc, pool_alloc_mode="queue") as tc:
        with (
            tc.tile_pool(name="const", bufs=1) as const,
            tc.tile_pool(name="dpool", bufs=2, space="DRAM") as dpool,
            tc.tile_pool(name="big", bufs=1) as big,
        ):
            # ---- big persistent tiles / input DMAs (x8 first, split) ----
            x8 = big.tile([P, KC, N], FP8, name="x8")
            x8r = x8d.rearrange("(o p) t -> p o t", p=P)
            for nt_ in range(NT):
                ts_ = slice(nt_ * 512, nt_ * 512 + 512)
                nc.sync.dma_start(out=x8[:, :, ts_], in_=x8r[:, :, ts_])
            wq_sb = big.tile([P, KC, D], FP8, name="wq_sb")
            nc.sync.dma_start(out=wq_sb[:],
                              in_=Wq8.rearrange("(o p) n -> p o n", p=P))
            wk_sb = big.tile([P, KC, D], FP8, name="wk_sb")
            nc.sync.dma_start(out=wk_sb[:],
                              in_=Wk8.rearrange("(o p) n -> p o n", p=P))
            wv_sb = big.tile([P, KC, D], FP8, name="wv_sb")
            nc.gpsimd.dma_start(out=wv_sb[:],
                                in_=Wv8.rearrange("(o p) n -> p o n", p=P))
            wo_sb = big.tile([P, KC, D], FP8, name="wo_sb")
            nc.gpsimd.dma_start(out=wo_sb[:],
                                in_=Wo8.rearrange("(o p) n -> p o n", p=P))
            w1_sb = big.tile([P, KC, DM], BF16, name="w1_sb")
            nc.gpsimd.dma_start(out=w1_sb[:],
                                in_=W1b.rearrange("(o p) n -> p o n", p=P))
            w2_sb = big.tile([P, MC1, D], FP8, name="w2_sb")
            nc.gpsimd.dma_start(out=w2_sb[:],
                                in_=W28.rearrange("(o p) n -> p o n", p=P))
            w2r_sb = big.tile([P, MC1, D], FP8, name="w2r_sb")
            nc.gpsimd.dma_start(out=w2r_sb[:],
                                in_=W28r.rearrange("(o p) n -> p o n", p=P))
            yr = big.tile([P, KC, TC], F32, name="yr")
            nc.gpsimd.dma_start(out=yr[:],
                                in_=yres.rearrange("(o p) t -> p o t", p=P))

            # ---- constants ----
            ident = const.tile([1, 1], F32)
            nc.vector.memset(ident[:], 1.0)
            eps1 = const.tile([1, 1], F32)
            nc.vector.memset(eps1[:], EPS)
            ones8 = const.tile([P, 2, 1], FP8, name="ones8")
            nc.vector.memset(ones8[:], 1.0 / D)      # 2^-9, exact in e4m3
            onec_f = const.tile([P, 1], F32)
            nc.vector.memset(onec_f[:], 1.0)
            inv8col = const.tile([P, 1], F32)
            nc.vector.memset(inv8col[:], 1.0 / S)
            onerow_f = const.tile([1, TC], F32)
            nc.vector.memset(onerow_f[:], 1.0)
            ones512 = const.tile([1, TC], F32R, name="ones512")
            nc.gpsimd.tensor_copy(out=ones512[:], in_=onerow_f[:])
            ones_row = const.tile([1, P], F32R, name="ones_row")
            nc.gpsimd.tensor_copy(out=ones_row[:], in_=onerow_f[:, 0:P])
            ones64 = const.tile([1, HD], F32R, name="ones64")
            nc.gpsimd.tensor_copy(out=ones64[:], in_=onerow_f[:, 0:HD])
            odiv_f = const.tile([P, 1], F32)
            nc.vector.memset(odiv_f[:], 1.0 / D)
            ones_div = const.tile([P, 1], F32R, name="ones_div")
            nc.gpsimd.tensor_copy(out=ones_div[:], in_=odiv_f[:])
            crow_st = const.tile([1, 4, D], F32, name="crow_st")
            nc.scalar.dma_start(out=crow_st[:],
                                in_=crow8.rearrange("(o r) d -> o r d", o=1))
            crow_sb = const.tile([1, 4, D], F32R, name="crow_sb")
            nc.gpsimd.tensor_copy(out=crow_sb[:], in_=crow_st[:])
            wkc_sb = const.tile([P, KC], F32, name="wkc_sb")
            nc.scalar.dma_start(out=wkc_sb[:], in_=wk8col[:])
            b2r_st = const.tile([1, D], F32, name="b2r_st")
            nc.scalar.dma_start(out=b2r_st[:], in_=b2row8[:])
            b2r_sb = const.tile([1, D], F32R, name="b2r_sb")
            nc.gpsimd.tensor_copy(out=b2r_sb[:], in_=b2r_st[:])
            ccol_sb = const.tile([P, KC + MC1], F32)
            nc.scalar.dma_start(out=ccol_sb[:], in_=ccol[:])

            KT = big.tile([P, KC, N], BF16, name="KT")
            QT = big.tile([P, KC, TC], BF16, name="QT")
            V8 = big.tile([P, JC, H, HD + 1], FP8, name="V8")
            st8a = big.tile([P, JC, 2, TC], FP8, name="st8a")
            st8b = big.tile([P, JC, 2, TC], FP8, name="st8b")
            RT = big.tile([P, KC, TC], FP8, name="RT")
            y2T = big.tile([P, KC, TC], F32R, name="y2T")
            mrep = big.tile([P, N], F32, name="mrep")
            arepq = big.tile([P, TC], F32, name="arepq")

            # ones-column of V8 (denominator trick)
            nc.vector.tensor_copy(out=V8[:, :, :, HD:HD + 1],
                                  in_=onec_f.to_broadcast((P, JC, H, 1)))

            a_stack = ExitStack()
            rowsA = a_stack.enter_context(tc.tile_pool(name="rowsA", bufs=1))
            sqp = a_stack.enter_context(tc.tile_pool(name="sqp", bufs=2))
            colsA = a_stack.enter_context(tc.tile_pool(name="colsA", bufs=1))
            rpool = a_stack.enter_context(tc.tile_pool(name="rpool", bufs=2))
            gemm_stack = ExitStack()
            gemm = gemm_stack.enter_context(
                tc.tile_pool(name="gemm", bufs=2, space="PSUM"))
            pss_stack = ExitStack()
            pss = pss_stack.enter_context(
                tc.tile_pool(name="pss", bufs=2, space="PSUM"))
            pstatA_stack = ExitStack()
            pstatA = pstatA_stack.enter_context(
                tc.tile_pool(name="pstatA", bufs=1, space="PSUM"))

            mean_row = rowsA.tile([1, N], F32R, name="mean_ln1")
            sd_row = rowsA.tile([1, 512], F32R, name="sd_ln1")
            rstd_row = rowsA.tile([1, N], F32, name="rstd_ln1")
            sdq_row = rowsA.tile([1, TC], F32R, name="sdq")
            r8row = rowsA.tile([1, TC], F32, name="r8row")
            rstd_tok = colsA.tile([P, JC], F32, name="rstd_tok")
            s64col = colsA.tile([P, JC], F32, name="s64col")
            rs8col = colsA.tile([P, JC], F32, name="rs8col")
            mean_dr = dpool.tile([1, N], F32, name="mean_dr", tag="md")
            rstd_dr = dpool.tile([1, TC], F32, name="rstd_dr", tag="rd")

            mean_f = mean_row.bitcast(F32)
            sd_f = sd_row.bitcast(F32)

            def ln1_tile(nt):
                """Emit stats for one 512-token tile."""
                ts = slice(nt * 512, nt * 512 + 512)
                pm = pstatA.tile([1, 512], F32, name="pm", tag="pmps")
                for a in range(2):
                    nc.tensor.matmul(pm[:], ones8[:],
                                     x8[:, 2 * a:2 * a + 2, ts],
                                     start=(a == 0), stop=(a == 1),
                                     perf_mode=DR, skip_group_check=True)
                nc.vector.tensor_copy(out=mean_row[:, ts], in_=pm[:])
                nc.sync.dma_start(out=mean_dr[:, ts], in_=mean_f[:, ts])
                nc.sync.dma_start(out=mrep[:, ts],
                                  in_=dram_bcast_src(mean_dr[:, ts], P))
                sq = sqp.tile([P, KC, 512], FP8, name="sq", tag="sq")
                eng = nc.vector if nt % 2 == 0 else nc.gpsimd
                eng.tensor_tensor(out=sq[:], in0=x8[:, :, ts],
                                  in1=x8[:, :, ts], op=OP.mult)
                ps_ = pstatA.tile([1, 512], F32, name="ps", tag="pmps")
                for a in range(2):
                    nc.tensor.matmul(ps_[:], ones8[:],
                                     sq[:, 2 * a:2 * a + 2, :],
                                     start=(a == 0), stop=(a == 1),
                                     perf_mode=DR, skip_group_check=True)
                nc.vector.tensor_tensor(out=sd_row[:], in0=mean_f[:, ts],
                                        in1=mean_f[:, ts], op=OP.mult)
                nc.vector.tensor_tensor(out=rstd_row[:, ts], in0=ps_[:],
                                        in1=sd_f[:], op=OP.subtract)
                nc.scalar.activation(out=sd_row.bitcast(F32)[:],
                                     in_=rstd_row[:, ts],
                                     func=AF.Sqrt, bias=eps1[:])
                nc.vector.reciprocal(out=rstd_row[:, ts], in_=sd_f[:])
                if nt == 0:
                    with nc.allow_low_precision(reason="f32r == f32 bits"):
                        nc.vector.tensor_copy(out=sdq_row[:], in_=sd_f[:])
                    nc.vector.tensor_scalar(
                        out=r8row[:], in0=rstd_row[:, 0:TC],
                        scalar1=1.0 / S, scalar2=None, op0=OP.mult)
                    nc.sync.dma_start(out=rstd_dr[:], in_=r8row[:])
                    nc.sync.dma_start(out=arepq[:],
                                      in_=dram_bcast_src(rstd_dr[:], P))
                pt = pstatA.tile([P, 4], F32, name="pt", tag="pt")
                for i in range(4):
                    jc = nt * 4 + i
                    nc.tensor.transpose(
                        pt[:, i:i + 1],
                        rstd_row[:, jc * P:(jc + 1) * P], ident[:])
                cs = slice(nt * 4, nt * 4 + 4)
                nc.vector.tensor_copy(out=rstd_tok[:, cs], in_=pt[:])
                nc.vector.tensor_scalar(out=s64col[:, cs],
                                        in0=rstd_tok[:, cs],
                                        scalar1=1.0 / (S * np.sqrt(HD)),
                                        scalar2=None, op0=OP.mult)
                nc.vector.tensor_scalar(out=rs8col[:, cs],
                                        in0=rstd_tok[:, cs],
                                        scalar1=1.0 / (S * S),
                                        scalar2=None, op0=OP.mult)

            # tile-0 stats up front (gates Q and the first scores)
            ln1_tile(0)

            # Q projection (own tokens; needs only tile-0 stats)
            for m in range(KC):
                ms = slice(m * P, m * P + P)
                pq = gemm.tile([P, 512], F32, name="pq", tag="pk")
                nc.tensor.matmul(pq[:], crow_sb[:, NSQ, ms],
                                 mean_row[:, 0:TC], start=True, stop=False,
                                 skip_group_check=True)
                nc.tensor.matmul(pq[:], crow_sb[:, BQ, ms], sdq_row[:],
                                 start=False, stop=False,
                                 skip_group_check=True)
                for a in range(2):
                    nc.tensor.matmul(pq[:], wq_sb[:, 2 * a:2 * a + 2, ms],
                                     x8[:, 2 * a:2 * a + 2, 0:TC],
                                     start=False, stop=(a == 1),
                                     perf_mode=DR, skip_group_check=True)
                nc.vector.tensor_tensor(out=QT[:, m, :], in0=pq[:],
                                        in1=arepq[:], op=OP.mult)

            # remaining LN1 stats (pipelines under the m=0 exp wave)
            for nt in range(1, NT):
                ln1_tile(nt)

            def k_proj(m):
                """K projection for feature chunk m (PE DRs + DVE STTs)."""
                ms = slice(m * P, m * P + P)
                for nt in range(NT):
                    ts = slice(nt * 512, nt * 512 + 512)
                    pk = gemm.tile([P, 512], F32, name="pk", tag="pk")
                    for a in range(2):
                        nc.tensor.matmul(pk[:],
                                         wk_sb[:, 2 * a:2 * a + 2, ms],
                                         x8[:, 2 * a:2 * a + 2, ts],
                                         start=(a == 0), stop=(a == 1),
                                         perf_mode=DR)
                    nc.vector.scalar_tensor_tensor(
                        out=KT[:, m, ts], in0=mrep[:, ts],
                        scalar=wkc_sb[:, m:m + 1],
                        in1=pk[:], op0=OP.mult, op1=OP.add)

            def v_chunk(jc):
                js = slice(jc * P, jc * P + P)
                pv = gemm.tile([P, 512], F32, name="pv", tag="pk")
                nc.tensor.matmul(pv[:], mean_row[:, js], crow_sb[:, NSV, :],
                                 start=True, stop=False,
                                 skip_group_check=True)
                for a in range(2):
                    nc.tensor.matmul(pv[:], x8[:, 2 * a:2 * a + 2, js],
                                     wv_sb[:, 2 * a:2 * a + 2, :],
                                     start=False, stop=(a == 1),
                                     perf_mode=DR, skip_group_check=True)
                # V8 = pv * rstd/64  (= v_hat / 8, fp8)
                nc.gpsimd.tensor_scalar(
                    out=V8[:, jc, :, 0:HD],
                    in0=pv.rearrange("p (h c) -> p h c", h=H),
                    scalar1=rs8col[:, jc:jc + 1], scalar2=None, op0=OP.mult)

            psr_stack = ExitStack()
            psr = [None]

            def av_head(m, r, last=False):
                """A@V for head 2m+r reading st8 buffer of wave m."""
                st8 = st8a if m % 2 == 0 else st8b
                h = 2 * m + r
                hs = slice(r * HD, r * HD + HD)
                pr = psr[0].tile([HD + 1, 512], F32, name="pr", tag="pr")
                for j in range(JC // 2):
                    nc.tensor.matmul(
                        pr[:], V8[:, 2 * j:2 * j + 2, h, :],
                        st8[:, 2 * j:2 * j + 2, r, :],
                        start=(j == 0), stop=(j == JC // 2 - 1),
                        perf_mode=DR)
                rs_row = rpool.tile([1, TC], F32R, name="rs_row", tag="rs")
                with nc.allow_low_precision(reason="f32r == f32 bits"):
                    nc.vector.reciprocal(out=rs_row[:], in_=pr[HD:HD + 1, :])
                if last:
                    # PE broadcast of the reciprocal row (cuts the DMA
                    # round-trip latency off the kernel tail)
                    rrep_p = gemm.tile([P, 512], F32, name="rrp", tag="pk")
                    nc.tensor.matmul(rrep_p[0:HD, :], ones64[:],
                                     rs_row[:], start=True, stop=True,
                                     skip_group_check=True)
                    nc.vector.tensor_tensor(out=RT[hs, m, :],
                                            in0=pr[0:HD, :],
                                            in1=rrep_p[0:HD, :], op=OP.mult)
                else:
                    rs_dr = dpool.tile([1, TC], F32, name="rs_dr", tag="rsd")
                    nc.sync.dma_start(out=rs_dr[:],
                                      in_=rs_row.bitcast(F32)[:])
                    rrep = rpool.tile([HD, TC], F32, name="rrep", tag="rrep")
                    nc.sync.dma_start(out=rrep[:],
                                      in_=dram_bcast_src(rs_dr[:], HD))
                    nc.vector.tensor_tensor(out=RT[hs, m, :],
                                            in0=pr[0:HD, :],
                                            in1=rrep[:], op=OP.mult)

            # per-m insert work lists: emitted one item per kv-chunk slot
            inserts = {
                0: [(lambda jc=jc: v_chunk(jc)) for jc in range(JC)]
                   + [lambda: k_proj(1)],
                1: [lambda: av_head(0, 0), lambda: av_head(0, 1),
                    lambda: k_proj(2)],
                2: [lambda: av_head(1, 0), lambda: av_head(1, 1),
                    lambda: k_proj(3)],
                3: [lambda: av_head(2, 0), lambda: av_head(2, 1)],
            }

            k_proj(0)
            for m in range(KC):
                if m == 1:
                    pstatA_stack.close()
                    psr[0] = psr_stack.enter_context(
                        tc.tile_pool(name="psr", bufs=2, space="PSUM"))
                st8 = st8a if m % 2 == 0 else st8b
                todo = list(inserts[m])
                for jc in range(JC):
                    js = slice(jc * P, jc * P + P)
                    psc = pss.tile([P, 1024], F32, name="psc", tag="psc")
                    for r in range(2):
                        nc.tensor.matmul(
                            psc[:, r * 512:r * 512 + 512],
                            KT[r * HD:r * HD + HD, m, js],
                            QT[r * HD:r * HD + HD, m, :],
                            start=True, stop=True)
                    nc.scalar.activation(
                        out=st8[:, jc, :, :], in_=psc[:],
                        func=AF.Exp, scale=s64col[:, jc:jc + 1])
                    if todo and (m > 0 or jc >= 1):
                        todo.pop(0)()
                for fn in todo:
                    fn()
            av_head(3, 0, last=True)
            av_head(3, 1, last=True)
            psr_stack.close()
            pss_stack.close()
            gemm_stack.close()
            a_stack.close()

            # ================= phase C =================
            with (
                tc.tile_pool(name="phc", bufs=1) as phc,
                tc.tile_pool(name="rowsC", bufs=1) as rowsC,
                tc.tile_pool(name="pmmC", bufs=2, space="PSUM") as pmmC,
            ):
                pstatC_stack = ExitStack()
                pstatC = pstatC_stack.enter_context(
                    tc.tile_pool(name="pstatC", bufs=2, space="PSUM"))
                y2f = y2T.bitcast(F32)
                mean2_row = rowsC.tile([1, TC], F32R, name="mean2")
                sd2_row = rowsC.tile([1, TC], F32R, name="sd2")
                rstd2_row = rowsC.tile([1, TC], F32R, name="rstd2")
                pm2 = pstatC.tile([1, 512], F32, name="pm2", tag="pmps")
                ps2 = pstatC.tile([1, 512], F32, name="ps2", tag="pmps")
                sq2 = phc.tile([P, KC, TC], FP8, name="sq2")
                for m in range(KC):
                    ms = slice(m * P, m * P + P)
                    po = pmmC.tile([P, 512], F32, name="po", tag="po")
                    for a in range(2):
                        nc.tensor.matmul(po[:], wo_sb[:, 2 * a:2 * a + 2, ms],
                                         RT[:, 2 * a:2 * a + 2, :],
                                         start=(a == 0), stop=(a == 1),
                                         perf_mode=DR)
                    nc.vector.scalar_tensor_tensor(
                        out=y2T[:, m, :], in0=po[:],
                        scalar=ccol_sb[:, m:m + 1],
                        in1=yr[:, m, :], op0=OP.add, op1=OP.add)
                    nc.tensor.matmul(pm2[:], ones_div[:], y2T[:, m, :],
                                     start=(m == 0), stop=(m == KC - 1))
                    nc.gpsimd.tensor_tensor(out=sq2[:, m, :],
                                            in0=y2f[:, m, :],
                                            in1=y2f[:, m, :], op=OP.mult)
                for a in range(2):
                    nc.tensor.matmul(ps2[:], ones8[:],
                                     sq2[:, 2 * a:2 * a + 2, :],
                                     start=(a == 0), stop=(a == 1),
                                     perf_mode=DR, skip_group_check=True)
                nc.vector.tensor_copy(out=mean2_row[:], in_=pm2[:])
                mean2_f = mean2_row.bitcast(F32)
                sd2_f = sd2_row.bitcast(F32)
                rstd2_f = rstd2_row.bitcast(F32)
                nc.vector.tensor_tensor(out=sd2_row[:], in0=mean2_f[:],
                                        in1=mean2_f[:], op=OP.mult)
                nc.vector.tensor_tensor(out=rstd2_row[:], in0=ps2[:],
                                        in1=sd2_f[:], op=OP.subtract)
                nc.scalar.activation(out=sd2_row.bitcast(F32)[:],
                                     in_=rstd2_f[:],
                                     func=AF.Sqrt, bias=eps1[:])
                with nc.allow_low_precision(reason="f32r == f32 bits"):
                    nc.vector.reciprocal(out=rstd2_row[:], in_=sd2_f[:])

                pmrep = pstatC.tile([P, 512], F32, name="pmrep", tag="rep")
                nc.tensor.matmul(pmrep[:], ones_row[:], mean2_row[:],
                                 start=True, stop=True)
                prrep = pstatC.tile([P, 512], F32, name="prrep", tag="rep")
                nc.tensor.matmul(prrep[:], ones_row[:], rstd2_row[:],
                                 start=True, stop=True)

                z2 = phc.tile([P, KC, TC], BF16, name="z2")
                zt = phc.tile([P, KC, TC], F32, name="zt")
                for k in range(KC):
                    eng = nc.vector if k % 2 == 0 else nc.gpsimd
                    eng.tensor_tensor(out=zt[:, k, :], in0=y2f[:, k, :],
                                      in1=pmrep[:], op=OP.subtract)
                    eng.tensor_tensor(out=z2[:, k, :], in0=zt[:, k, :],
                                      in1=prrep[:], op=OP.mult)

                pstatC_stack.close()
                # MLP: h2 accumulates on DoubleRow pairs as h1 chunks land
                h1 = phc.tile([P, MC1, TC], FP8, name="h1")
                ph2_stack = ExitStack()
                ph2 = ph2_stack.enter_context(
                    tc.tile_pool(name="ph2", bufs=1, space="PSUM"))
                p2s = [ph2.tile([P, 512], F32, name=f"p2_{m}", tag=f"p2_{m}")
                       for m in range(KC)]
                for m in range(KC):
                    ms = slice(m * P, m * P + P)
                    nc.tensor.matmul(p2s[m][:], b2r_sb[:, ms], ones512[:],
                                     start=True, stop=False,
                                     skip_group_check=True)
                for k in range(MC1):
                    ks = slice(k * P, k * P + P)
                    p1 = pmmC.tile([P, 512], F32, name="p1", tag="po")
                    for kk in range(KC):
                        nc.tensor.matmul(p1[:], w1_sb[:, kk, ks],
                                         z2[:, kk, :],
                                         start=(kk == 0), stop=(kk == KC - 1))
                    nc.scalar.activation(out=h1[:, k, :], in_=p1[:],
                                         func=AF.Gelu,
                                         bias=ccol_sb[:, KC + k:KC + k + 1])
                    if k % 2 == 1:
                        for m in range(KC):
                            ms = slice(m * P, m * P + P)
                            nc.tensor.matmul(
                                p2s[m][:], w2_sb[:, k - 1:k + 1, ms],
                                h1[:, k - 1:k + 1, :],
                                start=False, stop=False,
                                perf_mode=DR, skip_group_check=True)
                            nc.tensor.matmul(
                                p2s[m][:], w2r_sb[:, k - 1:k + 1, ms],
                                h1[:, k - 1:k + 1, :],
                                start=False, stop=(k == MC1 - 1),
                                perf_mode=DR, skip_group_check=True)

                out_sb = phc.tile([P, KC, TC], F32, name="out_sb")
                outT_r = outT.rearrange("(o p) t -> p o t", p=P)
                for m in range(KC):
                    nc.vector.scalar_tensor_tensor(
                        out=out_sb[:, m, :], in0=p2s[m][:],
                        scalar=inv8col[:],
                        in1=y2f[:, m, :], op0=OP.mult, op1=OP.add)
                    nc.sync.dma_start(out=outT_r[:, m, :],
                                      in_=out_sb[:, m, :])
                ph2_stack.close()

    return _finalize(nc) if do_finalize else nc


def prep_inputs(y, Wq, bq, Wk, bk, Wv, bv, Wo, bo, ln1_g, ln1_b, ln2_g, ln2_b,
                W1, b1, W2, b2):
    """Host-side weight folding + per-core input maps."""
    f = np.float32
    E4 = mybir.dt.np(FP8)
    Wq_ = (Wq * ln1_g[:, None]).astype(f)
    Wk_ = (Wk * ln1_g[:, None]).astype(f)
    Wv_ = (Wv * ln1_g[:, None]).astype(f)
    bq_ = (ln1_b @ Wq + bq).astype(f)
    bv_ = (ln1_b @ Wv + bv).astype(f)
    bo_ = (bv_ @ Wo + bo).astype(f)
    W1_ = (W1 * ln2_g[:, None]).astype(f)
    b1_ = (ln2_b @ W1 + b1).astype(f)

    crow8 = np.stack([S * bq_, S * (-Wq_.sum(0)), S * (-Wv_.sum(0)),
                      np.zeros(D, f)]).astype(f)
    wk8col = np.ascontiguousarray(
        (S * (-Wk_.sum(0))).reshape(KC, P).T).astype(f)
    b2row8 = (S * np.asarray(b2, f)).reshape(1, D).astype(f)
    ccol = np.concatenate([
        bo_.reshape(KC, P).T, b1_.reshape(MC1, P).T,
    ], axis=1).astype(f)

    W28 = (S * np.asarray(W2, f)).astype(E4)
    W28r = (S * np.asarray(W2, f) - W28.astype(f)).astype(E4)
    shared = {
        "Wq8": np.ascontiguousarray(S * Wq_).astype(E4),
        "Wk8": np.ascontiguousarray(S * Wk_).astype(E4),
        "Wv8": np.ascontiguousarray(S * Wv_).astype(E4),
        "Wo8": np.ascontiguousarray(S * np.asarray(Wo, f)).astype(E4),
        "W1b": np.ascontiguousarray(W1_).astype(mybir.dt.np(BF16)),
        "W28": np.ascontiguousarray(W28),
        "W28r": np.ascontiguousarray(W28r),
        "crow8": crow8, "wk8col": wk8col, "b2row8": b2row8, "ccol": ccol,
    }
    in_maps = []
    for c in range(8):
        b, s_ = divmod(c, 4)
        ts = s_ * TC
        yTm = np.asarray(y, np.float32)[b].T
        yrot = np.ascontiguousarray(np.roll(yTm, -ts, axis=1))
        in_maps.append({
            "x8d": yrot.astype(E4),
            "yres": np.ascontiguousarray(yrot[:, 0:TC]),
            **shared,
        })
    return in_maps


def gather_output(results):
    out = np.empty((B, N, D), np.float32)
    for c in range(8):
        b, s_ = divmod(c, 4)
        out[b, s_ * TC:(s_ + 1) * TC, :] = results[c]["outT"].T
    return out


_NC_CACHE = {}


def kernel(**inputs):
    """Full-input entry point: shard, run on 8 NeuronCores, gather."""
    from concourse.bass_utils import run_bass_kernel_spmd

    in_maps = prep_inputs(**{k: np.asarray(v) for k, v in inputs.items()})
    if "nc" not in _NC_CACHE:
        _NC_CACHE["nc"] = build_nc()
    nc = _NC_CACHE["nc"]
    res = run_bass_kernel_spmd(nc, in_maps, core_ids=list(range(8)))
    return gather_output(res.results)
